# revision 5
# baseline (speedup 1.0000x reference)
"""Trainium2 Bass kernel for nn_DyIntraModalityUpdate (dense transformer block).

Strategy: pure data-parallel over batch (B=32 -> 4 per core x 8 cores); each
core computes both the v- and q- branches for its batches. No collectives.

Per-core program (per branch, per batch):
  masked mean -> gate = sigmoid(relu(mean) @ w_gate + b)   [cross-branch]
  x^T via PE transpose (raw + relu copies, feature-major)
  QKV: k,q feature-major [dout, tok] (gate^2 folded into k, token-mask into q);
       v token-major [tok, dout] (bias via K=1 ones-row matmul)
  scores^T[j,i] per head (K=64, row-split pairs), exp via ACT with additive
       -1.25e8 key-mask bias (softmax without max-subtraction; fp32-safe here)
  row-sums replicated via all-ones lhsT matmul; reciprocal on DVE
  update^T = v^T@p (col-split head pairs into one PSUM tile), normalized and
       residual-added into x^T in place
  out = (x+u) @ w_out + b_out  (token-major PSUM, ACT evict, DMA out)

All matmuls run in float32r (TF32-like, 1 cycle/row at N>=256, ~1e-4 rel err).
"""
import os
import sys

import numpy as np

for _p in ("/opt/trn_rl_repo", "/root/.axon_site/_ro/trn_rl_repo"):
    if os.path.isdir(_p) and _p not in sys.path:
        sys.path.insert(0, _p)

import concourse.bass as bass  # noqa: E402
import concourse.mybir as mybir  # noqa: E402
import concourse.tile as tile  # noqa: E402
from concourse import bacc  # noqa: E402
from concourse.bass_utils import run_bass_kernel_spmd  # noqa: E402
from concourse.masks import make_identity  # noqa: E402

F32 = mybir.dt.float32
F32R = mybir.dt.float32r
ALU = mybir.AluOpType
ACTF = mybir.ActivationFunctionType

B_CORE = 4
NTOK = 256
D = 1024
DQKV = 3 * D
NCORES = 8
NEGBIAS = -1e9 / 8.0  # masked_fill(-1e9) then /sqrt(64)

WEIGHT_NAMES = ("w_v4q", "b_v4q", "w_q4v", "b_q4v",
                "w_vlin", "b_vlin", "w_qlin", "b_qlin",
                "w_vout", "b_vout", "w_qout", "b_qout")


def build_nc():
    nc = bacc.Bacc("TRN2", target_bir_lowering=False, debug=False)
    dram = {}

    def din(name, shape):
        dram[name] = nc.dram_tensor(name, shape, F32, kind="ExternalInput").ap()

    def dout(name, shape):
        dram[name] = nc.dram_tensor(name, shape, F32, kind="ExternalOutput").ap()

    din("v", [B_CORE, NTOK, D])
    din("q", [B_CORE, NTOK, D])
    din("v_mask", [B_CORE, NTOK])
    din("q_mask", [B_CORE, NTOK])
    for g in ("v4q", "q4v"):
        din(f"w_{g}", [D, D])
        din(f"b_{g}", [D])
    for x in ("v", "q"):
        din(f"w_{x}lin", [D, DQKV])
        din(f"b_{x}lin", [DQKV])
        din(f"w_{x}out", [D, D])
        din(f"b_{x}out", [D])
    dout("out_v", [B_CORE, NTOK, D])
    dout("out_q", [B_CORE, NTOK, D])

    with tile.TileContext(nc) as tc:
        with tc.tile_pool(name="cpool", bufs=1) as cpool, \
             tc.tile_pool(name="wpool", bufs=1) as wpool, \
             tc.tile_pool(name="pspool", bufs=7, space="PSUM") as ps:
            # ---- constants ----
            ones_f = cpool.tile([128, 128], F32, name="ones_f")
            nc.gpsimd.memset(ones_f[:], 1.0)
            ones128 = cpool.tile([128, 128], F32R, name="ones128")
            nc.vector.tensor_copy(ones128[:], ones_f[:])
            ones1 = cpool.tile([1, 128], F32R, name="ones1")
            nc.vector.tensor_copy(ones1[:], ones_f[0:1, :])
            ident_f = cpool.tile([128, 128], F32, name="ident_f")
            make_identity(nc, ident_f[:])
            ident = cpool.tile([128, 128], F32R, name="ident")
            nc.vector.tensor_copy(ident[:], ident_f[:])
            zero_f = cpool.tile([128, 4], F32, name="zero_f")
            nc.gpsimd.memset(zero_f[:], 0.0)

            meanT = {}
            g2T = {}
            # ---- prologue: masked means ----
            with tc.tile_pool(name="propool", bufs=1) as pp:
                for X in ("v", "q"):
                    x_d = dram[X]
                    m_d = dram[f"{X}_mask"]
                    ps_mean = [ps.tile([4, 512], F32, name=f"psmean_{X}{h}", tag="ps")
                               for h in range(2)]
                    ps_n = ps.tile([4, 2], F32, name=f"psn_{X}", tag="ps")
                    for b in range(B_CORE):
                        for jt in range(2):
                            xt = pp.tile([128, D], F32R, name=f"mx_{X}_{b}_{jt}",
                                         tag="mx", bufs=3)
                            nc.sync.dma_start(
                                xt[:], x_d[b, jt * 128:(jt + 1) * 128, :].bitcast(F32R))
                            mc = pp.tile([128, 4], F32R, name=f"mc_{X}_{b}_{jt}",
                                         tag="mc", bufs=4)
                            nc.vector.tensor_copy(mc[:], zero_f[:])
                            nc.sync.dma_start(
                                mc[:, b:b + 1],
                                m_d[b, jt * 128:(jt + 1) * 128].bitcast(F32R).unsqueeze(1))
                            first = (b == 0 and jt == 0)
                            last = (b == B_CORE - 1 and jt == 1)
                            for h in range(2):
                                nc.tensor.matmul(ps_mean[h][:], mc[:],
                                                 xt[:, h * 512:(h + 1) * 512],
                                                 start=first, stop=last)
                            nc.tensor.matmul(ps_n[:], mc[:], ones128[:, 0:2],
                                             start=first, stop=last)
                    recn = pp.tile([4, 1], F32, name=f"recn_{X}", tag="recn", bufs=2)
                    nc.vector.reciprocal(recn[:], ps_n[:, 0:1])
                    rmean = pp.tile([4, D], F32R, name=f"rmean_{X}", tag="rmean", bufs=2)
                    for h in range(2):
                        # relu(masked_sum / n): (psum * recn) max 0
                        nc.vector.tensor_scalar(rmean[:, h * 512:(h + 1) * 512],
                                                ps_mean[h][:], recn[:], 0.0,
                                                ALU.mult, ALU.max)
                    mt = wpool.tile([128, 8, 4], F32R, name=f"meanT_{X}")
                    for c in range(8):
                        pst = ps.tile([128, 4], F32R, name=f"psmt_{X}{c}", tag="ps")
                        nc.tensor.transpose(pst[:], rmean[:, c * 128:(c + 1) * 128],
                                            ident[0:4, 0:4])
                        nc.vector.tensor_copy(mt[:, c, :], pst[:])
                    meanT[X] = mt

                # ---- prologue: gates ----
                # v4q gate (from v-mean) scales the q branch; q4v scales v.
                for gname, dst in (("v4q", "q"), ("q4v", "v")):
                    w_d = dram[f"w_{gname}"]
                    b_d = dram[f"b_{gname}"]
                    src = meanT["v" if gname == "v4q" else "q"]
                    wg = pp.tile([128, 8, D], F32R, name=f"wg_{gname}", tag="wg", bufs=1)
                    for kt in range(8):
                        nc.sync.dma_start(
                            wg[:, kt, :], w_d[kt * 128:(kt + 1) * 128, :].bitcast(F32R))
                    bg = pp.tile([1, D], F32R, name=f"bg_{gname}", tag="bg", bufs=1)
                    nc.sync.dma_start(bg[:], b_d.bitcast(F32R).unsqueeze(0))
                    gsb = pp.tile([4, D], F32, name=f"g_{gname}", tag="gsb", bufs=1)
                    for h in range(2):
                        psg = ps.tile([4, 512], F32, name=f"psg_{gname}{h}", tag="ps")
                        for kt in range(8):
                            nc.tensor.matmul(psg[:], src[:, kt, :],
                                             wg[:, kt, h * 512:(h + 1) * 512],
                                             start=(kt == 0), stop=False)
                        nc.tensor.matmul(psg[:], ones1[0:1, 0:4],
                                         bg[:, h * 512:(h + 1) * 512],
                                         start=False, stop=True)
                        nc.scalar.activation(gsb[:, h * 512:(h + 1) * 512], psg[:],
                                             ACTF.Sigmoid)
                    g1 = pp.tile([4, D], F32, name=f"g1_{gname}", tag="g1", bufs=1)
                    nc.vector.tensor_scalar_add(g1[:], gsb[:], 1.0)
                    g2 = pp.tile([4, D], F32R, name=f"g2_{gname}", tag="g2", bufs=1)
                    nc.vector.tensor_tensor(g2[:], g1[:], g1[:], ALU.mult)
                    gt = wpool.tile([128, 8, 4], F32, name=f"g2T_{dst}")
                    for c in range(8):
                        pst = ps.tile([128, 4], F32R, name=f"psgt_{gname}{c}", tag="ps")
                        nc.tensor.transpose(pst[:], g2[:, c * 128:(c + 1) * 128],
                                            ident[0:4, 0:4])
                        nc.vector.tensor_copy(gt[:, c, :], pst[:])
                    g2T[dst] = gt

            # ---- main: per branch ----
            main_ctx = tc.tile_pool(name="wopool", bufs=3)
            wopool = main_ctx.__enter__()
            apool_ctx = tc.tile_pool(name="apool", bufs=1)
            apool = apool_ctx.__enter__()
            for X in ("v", "q"):
                gate = g2T[X]
                x_d = dram[X]
                m_d = dram[f"{X}_mask"]
                wlin_d = dram[f"w_{X}lin"]
                blin_d = dram[f"b_{X}lin"]
                wout_d = dram[f"w_{X}out"]
                bout_d = dram[f"b_{X}out"]
                out_d = dram[f"out_{X}"]

                wl = []
                for kt in range(8):
                    t = wpool.tile([128, DQKV], F32R, name=f"wl_{X}_{kt}",
                                   tag=f"wl{kt}", bufs=1)
                    nc.sync.dma_start(
                        t[:], wlin_d[kt * 128:(kt + 1) * 128, :].bitcast(F32R))
                    wl.append(t)
                b_kq = wpool.tile([128, 16], F32, name=f"bkq_{X}", tag="bkq", bufs=2)
                nc.sync.dma_start(b_kq[:],
                                  blin_d[0:2048].rearrange("(o p) -> p o", p=128))
                b_v = wpool.tile([1, D], F32R, name=f"bv_{X}", tag="bv", bufs=2)
                nc.sync.dma_start(b_v[:], blin_d[2048:3072].bitcast(F32R).unsqueeze(0))
                b_o = wpool.tile([1, D], F32R, name=f"bo_{X}", tag="bo", bufs=2)
                nc.sync.dma_start(b_o[:], bout_d.bitcast(F32R).unsqueeze(0))

                for b in range(B_CORE):
                    # loads
                    xt = []
                    for jt in range(2):
                        t = apool.tile([128, D], F32R, name=f"x_{X}_{b}_{jt}",
                                       tag="xt", bufs=2)
                        nc.sync.dma_start(
                            t[:], x_d[b, jt * 128:(jt + 1) * 128, :].bitcast(F32R))
                        xt.append(t)
                    mrow = apool.tile([1, NTOK], F32R, name=f"mrow_{X}_{b}",
                                      tag="mrow", bufs=2)
                    nc.sync.dma_start(mrow[:], m_d[b].bitcast(F32R).unsqueeze(0))
                    psmr = ps.tile([128, NTOK], F32, name=f"psmr_{X}_{b}", tag="ps")
                    nc.tensor.matmul(psmr[:], ones1[:], mrow[:], start=True, stop=True)
                    maskrep = apool.tile([128, NTOK], F32, name=f"maskrep_{X}_{b}",
                                         tag="maskrep", bufs=2)
                    nc.vector.tensor_copy(maskrep[:], psmr[:])
                    mb = []
                    for jt in range(2):
                        mcol = apool.tile([128, 1], F32, name=f"mcol_{X}_{b}_{jt}",
                                          tag="mcol", bufs=4)
                        nc.sync.dma_start(
                            mcol[:], m_d[b, jt * 128:(jt + 1) * 128].unsqueeze(1))
                        t = apool.tile([128, 1], F32, name=f"mbias_{X}_{b}_{jt}",
                                       tag="mbias", bufs=4)
                        nc.vector.tensor_scalar(t[:], mcol[:], 1.0, -NEGBIAS,
                                                ALU.subtract, ALU.mult)
                        mb.append(t)

                    # transpose x -> feature-major raw + relu copies
                    xTraw = apool.tile([128, 8, NTOK], F32R, name=f"xTraw_{X}_{b}",
                                       tag="xTraw", bufs=1)
                    xTrelu = apool.tile([128, 8, NTOK], F32R, name=f"xTrelu_{X}_{b}",
                                        tag="xTrelu", bufs=1)
                    for jt in range(2):
                        for c in range(8):
                            pst = ps.tile([128, 128], F32R,
                                          name=f"pstp_{X}_{b}_{jt}_{c}", tag="ps")
                            nc.tensor.transpose(pst[:], xt[jt][:, c * 128:(c + 1) * 128],
                                                ident[:])
                            nc.vector.tensor_copy(
                                xTraw[:, c, jt * 128:(jt + 1) * 128], pst[:])
                            nc.scalar.activation(
                                xTrelu[:, c, jt * 128:(jt + 1) * 128], pst[:], ACTF.Relu)

                    # v projection: token-major [tok, dout], bias via ones-row matmul
                    vtok = []
                    for jt in range(2):
                        vt = apool.tile([128, D], F32R, name=f"vtok_{X}_{b}_{jt}",
                                        tag=f"vtok{jt}", bufs=1)
                        vtok.append(vt)
                    for jt in range(2):
                        for ch in range(2):
                            psv = ps.tile([128, 512], F32,
                                          name=f"psv_{X}_{b}_{jt}_{ch}", tag="ps")
                            for kt in range(8):
                                nc.tensor.matmul(
                                    psv[:], xTrelu[:, kt, jt * 128:(jt + 1) * 128],
                                    wl[kt][:, 2048 + ch * 512:2048 + (ch + 1) * 512],
                                    start=(kt == 0), stop=False)
                            nc.tensor.matmul(psv[:], ones1[:],
                                             b_v[:, ch * 512:(ch + 1) * 512],
                                             start=False, stop=True)
                            nc.scalar.copy(vtok[jt][:, ch * 512:(ch + 1) * 512], psv[:])

                    # k,q projections + attention, one 128-feature head-pair at a time
                    for mp in range(8):
                        kq = {}
                        for part in (mp, 8 + mp):  # k chunk then q chunk
                            psq = ps.tile([128, NTOK], F32,
                                          name=f"pskq_{X}_{b}_{part}", tag="ps")
                            for kt in range(8):
                                nc.tensor.matmul(
                                    psq[:], wl[kt][:, part * 128:(part + 1) * 128],
                                    xTrelu[:, kt, :], start=(kt == 0), stop=(kt == 7))
                            if part < 8:
                                t = apool.tile([128, NTOK], F32R,
                                               name=f"k_{X}_{b}_{mp}", tag=f"k{mp}",
                                               bufs=1)
                                # (psum + bias) * (1+gate)^2   [both per-partition]
                                nc.vector.tensor_scalar(
                                    t[:], psq[:], b_kq[:, part:part + 1],
                                    gate[:, part, b:b + 1], ALU.add, ALU.mult)
                                k_t = t
                            else:
                                t = apool.tile([128, NTOK], F32R,
                                               name=f"q_{X}_{b}_{mp}", tag=f"q{mp}",
                                               bufs=1)
                                # (psum + bias) * token_mask  [mask replicated]
                                nc.vector.scalar_tensor_tensor(
                                    t[:], psq[:], b_kq[:, part:part + 1], maskrep[:],
                                    ALU.add, ALU.mult)
                                q_t = t
                        del kq

                        # scores^T + exp, per key-token-tile jt
                        pT_mp = []
                        for jt in range(2):
                            pt2 = apool.tile([128, 512], F32R,
                                             name=f"pT_{X}_{b}_{mp}_{jt}", tag="pT",
                                             bufs=4)
                            for h_loc in range(2):
                                r0 = h_loc * 64
                                pss = ps.tile([128, NTOK], F32,
                                              name=f"pss_{X}_{b}_{mp}_{jt}_{h_loc}",
                                              tag="ps")
                                nc.tensor.matmul(
                                    pss[:], k_t[r0:r0 + 64, jt * 128:(jt + 1) * 128],
                                    q_t[r0:r0 + 64, :], start=True, stop=True)
                                nc.scalar.activation(
                                    pt2[:, h_loc * 256:(h_loc + 1) * 256], pss[:],
                                    ACTF.Exp, bias=mb[jt][:], scale=0.125)
                            pT_mp.append(pt2)

                        # replicated row-sums + reciprocal
                        psr = ps.tile([128, 512], F32, name=f"psr_{X}_{b}_{mp}",
                                      tag="ps")
                        nc.tensor.matmul(psr[:], ones128[:], pT_mp[0][:],
                                         start=True, stop=False)
                        nc.tensor.matmul(psr[:], ones128[:], pT_mp[1][:],
                                         start=False, stop=True)
                        rinv = apool.tile([128, 512], F32, name=f"rinv_{X}_{b}_{mp}",
                                          tag="rinv", bufs=2)
                        nc.vector.reciprocal(rinv[:], psr[:])

                        # update^T = v^T @ p (one [64,256] psum per head;
                        # partition-shifted DVE eviction into the pair tile)
                        u_tmp = apool.tile([128, NTOK], F32, name=f"ut_{X}_{b}_{mp}",
                                           tag="utmp", bufs=2)
                        for h_loc in range(2):
                            h = 2 * mp + h_loc
                            psu = ps.tile([64, NTOK], F32,
                                          name=f"psu_{X}_{b}_{mp}_{h_loc}", tag="ps")
                            for jt in range(2):
                                nc.tensor.matmul(
                                    psu[:],
                                    vtok[jt][:, h * 64:(h + 1) * 64],
                                    pT_mp[jt][:, h_loc * 256:(h_loc + 1) * 256],
                                    start=(jt == 0), stop=(jt == 1))
                            r0 = h_loc * 64
                            nc.vector.tensor_tensor(
                                u_tmp[r0:r0 + 64, :], psu[0:64, :],
                                rinv[0:64, h_loc * 256:(h_loc + 1) * 256],
                                ALU.mult)
                        # residual: x^T += u^T (in place)
                        nc.vector.tensor_tensor(xTraw[:, mp, :], xTraw[:, mp, :],
                                                u_tmp[:], ALU.add)

                    # output projection (w_out streamed), PSUM -> ACT -> DMA
                    pso = [ps.tile([128, 512], F32, name=f"pso_{X}_{b}_{i}", tag="ps")
                           for i in range(4)]
                    for kt in range(8):
                        wo = wopool.tile([128, D], F32R, name=f"wo_{X}_{b}_{kt}",
                                         tag="wo")
                        nc.sync.dma_start(
                            wo[:], wout_d[kt * 128:(kt + 1) * 128, :].bitcast(F32R))
                        for i in range(4):
                            it, ch = divmod(i, 2)
                            nc.tensor.matmul(pso[i][:],
                                             xTraw[:, kt, it * 128:(it + 1) * 128],
                                             wo[:, ch * 512:(ch + 1) * 512],
                                             start=(kt == 0), stop=False)
                    for i in range(4):
                        it, ch = divmod(i, 2)
                        nc.tensor.matmul(pso[i][:], ones1[:],
                                         b_o[:, ch * 512:(ch + 1) * 512],
                                         start=False, stop=True)
                        osb = apool.tile([128, 512], F32, name=f"osb_{X}_{b}_{i}",
                                         tag="osb", bufs=2)
                        nc.scalar.copy(osb[:], pso[i][:])
                        nc.sync.dma_start(
                            out_d[b, it * 128:(it + 1) * 128, ch * 512:(ch + 1) * 512],
                            osb[:])
            apool_ctx.__exit__(None, None, None)
            main_ctx.__exit__(None, None, None)
    nc.compile()
    return nc


_NC = None


def _get_nc():
    global _NC
    if _NC is None:
        _NC = build_nc()
    return _NC


def run(inputs, trace=False):
    nc = _get_nc()
    in_maps = []
    for c in range(NCORES):
        sl = slice(B_CORE * c, B_CORE * (c + 1))
        m = {"v": np.ascontiguousarray(np.asarray(inputs["v"], dtype=np.float32)[sl]),
             "q": np.ascontiguousarray(np.asarray(inputs["q"], dtype=np.float32)[sl]),
             "v_mask": np.ascontiguousarray(
                 np.asarray(inputs["v_mask"], dtype=np.float32)[sl]),
             "q_mask": np.ascontiguousarray(
                 np.asarray(inputs["q_mask"], dtype=np.float32)[sl])}
        for name in WEIGHT_NAMES:
            m[name] = np.ascontiguousarray(np.asarray(inputs[name], dtype=np.float32))
        in_maps.append(m)
    res = run_bass_kernel_spmd(nc, in_maps, core_ids=list(range(NCORES)),
                               trace=trace)
    uv = np.concatenate([res.results[c]["out_v"] for c in range(NCORES)], axis=0)
    uq = np.concatenate([res.results[c]["out_q"] for c in range(NCORES)], axis=0)
    return (uv, uq), res


def kernel(**inputs):
    (uv, uq), _ = run(inputs, trace=False)
    return uv, uq
